# revision 5
# baseline (speedup 1.0000x reference)
"""MoE (16 experts, top-2) expert-parallel kernel for 8 TRN2 NeuronCores.

Strategy:
  - Gating (logits -> top-2 -> softmax) is computed with jnp on the default
    jax backend, mirroring the reference ops exactly so near-tie tokens route
    identically.
  - Tokens are dispatched per expert on the host (gather + transpose), padded
    to a per-slot capacity. Experts are paired big+small by routed count and
    one pair is assigned per core (slot A = big, slot B = small), so all cores
    do near-identical work.
  - Each core runs a Bass/Tile kernel computing, per expert,
        yT = (relu(W1.T @ xgT + b1).T @ W2).T   (out_d x tokens layout)
    with float32r matmuls (full PE rate, ~1e-3 rel err) accumulating in fp32.
    Weights stream through SBUF in hid-groups of 512, double-buffered; y is
    accumulated in SBUF and evicted (+b2) during the last hid-group.
  - Host applies the routing weight and scatter-adds per expert into the
    full [B, D_OUT] output.
"""

import numpy as np

NUM_EXPERTS = 16
TOP_K = 2
D_IN = 1024
D_HID = 4096
D_OUT = 1024
BATCH = 8192
N_CORES = 8
EPC = NUM_EXPERTS // N_CORES  # experts per core

HG = 512                      # hid group size streamed per weight block
N_GROUPS = D_HID // HG        # 8
KT1 = D_IN // 128             # 8  k-tiles for mm1
KT2 = HG // 128               # 4  k-tiles per group for mm2
MT1 = HG // 128               # 4  hid m-tiles per group
MT2 = D_OUT // 128            # 8  out m-tiles

_last_run_info = {}


def _round_cap(n):
    return ((n + 63) // 64) * 64


def _token_tiles(C):
    """Split capacity C into moving-dim tiles, each in [256, 512]."""
    tiles = []
    t0 = 0
    while t0 < C:
        rem = C - t0
        if rem <= 512:
            tn = rem
        elif rem <= 768:
            tn = rem - 256
        else:
            tn = 512
        tiles.append((t0, tn))
        t0 += tn
    assert all(256 <= tn <= 512 for _, tn in tiles), (C, tiles)
    return tiles


def _build_program(CA, CB):
    from concourse import bacc, mybir, tile

    f32 = mybir.dt.float32
    f32r = mybir.dt.float32r

    nc = bacc.Bacc("TRN2", target_bir_lowering=False, debug=False)
    caps = [CA, CB]
    xgT = [
        nc.dram_tensor(f"xgT{s}", [D_IN, caps[s]], f32r, kind="ExternalInput")
        for s in range(EPC)
    ]
    yT = [
        nc.dram_tensor(f"yT{s}", [D_OUT, caps[s]], f32, kind="ExternalOutput")
        for s in range(EPC)
    ]
    w1 = nc.dram_tensor("w1", [EPC * D_IN, D_HID], f32r, kind="ExternalInput")
    w2 = nc.dram_tensor("w2", [EPC * D_HID, D_OUT], f32r, kind="ExternalInput")
    b1 = nc.dram_tensor("b1", [128, EPC * (D_HID // 128)], f32, kind="ExternalInput")
    b2 = nc.dram_tensor("b2", [128, EPC * (D_OUT // 128)], f32, kind="ExternalInput")

    with tile.TileContext(nc) as tc:
        with (
            tc.tile_pool(name="xg", bufs=1) as xg_pool,
            tc.tile_pool(name="wt", bufs=2) as wt_pool,
            tc.tile_pool(name="h", bufs=2) as h_pool,
            tc.tile_pool(name="yacc", bufs=1) as y_pool,
            tc.tile_pool(name="stage", bufs=3) as st_pool,
            tc.tile_pool(name="const", bufs=1) as c_pool,
            tc.tile_pool(name="ph", bufs=3, space="PSUM") as ph_pool,
            tc.tile_pool(name="py", bufs=3, space="PSUM") as py_pool,
        ):
            b1_sb = c_pool.tile([128, EPC * (D_HID // 128)], f32, tag="b1")
            b2_sb = c_pool.tile([128, EPC * (D_OUT // 128)], f32, tag="b2")
            nc.gpsimd.dma_start(b1_sb[:], b1.ap())
            nc.gpsimd.dma_start(b2_sb[:], b2.ap())

            for e in range(EPC):
                C = caps[e]
                ttiles = _token_tiles(C)
                if e == 0:
                    # First weight blocks issue before the token chunks so
                    # the two DMA rings each carry one critical block and
                    # the PE can start ~10us in.
                    w1_g0 = wt_pool.tile([128, KT1, HG], f32r, tag="w1g")
                    nc.gpsimd.dma_start(
                        w1_g0[:],
                        w1.ap()[0:D_IN, 0:HG].rearrange(
                            "(kt p) h -> p kt h", p=128
                        ),
                    )
                    w2_g0 = wt_pool.tile([128, KT2, D_OUT], f32r, tag="w2g")
                    nc.gpsimd.dma_start(
                        w2_g0[:],
                        w2.ap()[0:HG, :].rearrange("(kt p) o -> p kt o", p=128),
                    )
                xg_sb = xg_pool.tile([128, KT1, C], f32r, tag=f"xg{e}")
                for (t0, tn) in ttiles:
                    nc.sync.dma_start(
                        xg_sb[:, :, t0:t0 + tn],
                        xgT[e].ap()[:, t0:t0 + tn].rearrange(
                            "(kt p) c -> p kt c", p=128
                        ),
                    )
                y_acc = y_pool.tile([128, MT2, CA], f32, tag="yacc")

                for g in range(N_GROUPS):
                    if e == 0 and g == 0:
                        w1_g, w2_g = w1_g0, w2_g0
                    else:
                        w1_g = wt_pool.tile([128, KT1, HG], f32r, tag="w1g")
                        nc.gpsimd.dma_start(
                            w1_g[:],
                            w1.ap()[e * D_IN:(e + 1) * D_IN, g * HG:(g + 1) * HG]
                            .rearrange("(kt p) h -> p kt h", p=128),
                        )
                        w2_g = wt_pool.tile([128, KT2, D_OUT], f32r, tag="w2g")
                        nc.gpsimd.dma_start(
                            w2_g[:],
                            w2.ap()[e * D_HID + g * HG: e * D_HID + (g + 1) * HG, :]
                            .rearrange("(kt p) o -> p kt o", p=128),
                        )

                    for (t0, tn) in ttiles:
                        hs = []
                        for m in range(MT1):
                            ps_h = ph_pool.tile([128, 512], f32, tag="ph")
                            for kt in range(KT1):
                                nc.tensor.matmul(
                                    ps_h[:, :tn],
                                    w1_g[:, kt, m * 128:(m + 1) * 128],
                                    xg_sb[:, kt, t0:t0 + tn],
                                    start=(kt == 0),
                                    stop=(kt == KT1 - 1),
                                )
                            h_m = h_pool.tile([128, 512], f32r, tag=f"h{m}")
                            gm = g * MT1 + m
                            nc.scalar.activation(
                                h_m[:, :tn],
                                ps_h[:, :tn],
                                mybir.ActivationFunctionType.Relu,
                                bias=b1_sb[
                                    :, e * (D_HID // 128) + gm:
                                    e * (D_HID // 128) + gm + 1
                                ],
                            )
                            hs.append(h_m)
                        for mo in range(MT2):
                            ps_y = py_pool.tile([128, 512], f32, tag="py")
                            for k2 in range(KT2):
                                nc.tensor.matmul(
                                    ps_y[:, :tn],
                                    w2_g[:, k2, mo * 128:(mo + 1) * 128],
                                    hs[k2][:, :tn],
                                    start=(k2 == 0),
                                    stop=(k2 == KT2 - 1),
                                )
                            if g == 0:
                                nc.vector.tensor_copy(
                                    y_acc[:, mo, t0:t0 + tn], ps_y[:, :tn]
                                )
                            else:
                                nc.vector.tensor_add(
                                    y_acc[:, mo, t0:t0 + tn],
                                    y_acc[:, mo, t0:t0 + tn],
                                    ps_y[:, :tn],
                                )
                        if g == N_GROUPS - 1:
                            # Evict this token tile (+b2) while later tiles
                            # of the last group still compute.
                            for mo in range(MT2):
                                stage = st_pool.tile([128, 512], f32, tag="stage")
                                nc.scalar.activation(
                                    stage[:, :tn],
                                    y_acc[:, mo, t0:t0 + tn],
                                    mybir.ActivationFunctionType.Identity,
                                    bias=b2_sb[
                                        :, e * (D_OUT // 128) + mo:
                                        e * (D_OUT // 128) + mo + 1
                                    ],
                                )
                                nc.sync.dma_start(
                                    yT[e].ap()[mo * 128:(mo + 1) * 128, t0:t0 + tn],
                                    stage[:, :tn],
                                )
    nc.compile()
    return nc


def _gating(x, Wg):
    """Mirror the reference gating ops on the default jax backend."""
    import jax
    import jax.numpy as jnp

    logits = jnp.asarray(x) @ jnp.asarray(Wg)
    top_vals, top_idx = jax.lax.top_k(logits, TOP_K)
    routing_weights = jax.nn.softmax(top_vals, axis=-1)
    return np.asarray(top_idx), np.asarray(routing_weights)


def kernel(x, Wg, W1, b1, W2, b2):
    from concourse.bass_utils import run_bass_kernel_spmd

    x = np.ascontiguousarray(np.asarray(x, dtype=np.float32))
    Wg = np.asarray(Wg, dtype=np.float32)
    W1 = np.asarray(W1, dtype=np.float32)
    b1 = np.asarray(b1, dtype=np.float32)
    W2 = np.asarray(W2, dtype=np.float32)
    b2 = np.asarray(b2, dtype=np.float32)

    top_idx, routing_w = _gating(x, Wg)

    # Per-expert token lists (ascending token order) and routing weights.
    idx_lists, w_lists = [], []
    for e in range(NUM_EXPERTS):
        sel = top_idx == e  # [B, k] bool
        tok = np.nonzero(sel.any(axis=1))[0]
        slot = sel[tok].argmax(axis=1)
        idx_lists.append(tok)
        w_lists.append(routing_w[tok, slot].astype(np.float32))

    # Pair big+small experts; pair i -> core i, slot 0 = big, slot 1 = small.
    counts = np.array([len(t) for t in idx_lists])
    order = np.argsort(-counts, kind="stable")
    pair_experts = [
        (int(order[i]), int(order[NUM_EXPERTS - 1 - i])) for i in range(N_CORES)
    ]
    CA = _round_cap(max(counts[order[:N_CORES]]))
    CB = _round_cap(max(counts[order[N_CORES:]]))
    caps = [CA, CB]

    xT = np.ascontiguousarray(x.T)  # [D_IN, B]

    in_maps = []
    for c in range(N_CORES):
        im = {}
        es = pair_experts[c]
        for s, e in enumerate(es):
            tok = idx_lists[e]
            xgT = np.zeros((D_IN, caps[s]), dtype=np.float32)
            xgT[:, : len(tok)] = xT[:, tok]
            im[f"xgT{s}"] = xgT
        im["w1"] = np.ascontiguousarray(W1[list(es)]).reshape(EPC * D_IN, D_HID)
        im["w2"] = np.ascontiguousarray(W2[list(es)]).reshape(EPC * D_HID, D_OUT)
        im["b1"] = np.ascontiguousarray(
            b1[list(es)].reshape(EPC * (D_HID // 128), 128).T
        )
        im["b2"] = np.ascontiguousarray(
            b2[list(es)].reshape(EPC * (D_OUT // 128), 128).T
        )
        in_maps.append(im)

    nc = _build_program(CA, CB)
    res = run_bass_kernel_spmd(nc, in_maps, core_ids=list(range(N_CORES)))
    _last_run_info["results"] = res

    out = np.zeros((BATCH, D_OUT), dtype=np.float32)
    for e in range(NUM_EXPERTS):
        c = next(i for i, p in enumerate(pair_experts) if e in p)
        s = pair_experts[c].index(e)
        tok = idx_lists[e]
        if len(tok) == 0:
            continue
        yT_e = res.results[c][f"yT{s}"][:, : len(tok)]
        out[tok] += w_lists[e][:, None] * yT_e.T
    return out
